# revision 40
# baseline (speedup 1.0000x reference)
"""SAM-style global attention (1,64,64,768), 12 heads, on 8 TRN2 NeuronCores.

Sharding: 24 units of (head, query-half-of-2048). Core c owns units
[3c, 3c+3) = 1.5 heads of queries spanning exactly 2 heads. Each core emits a
partial projected output outT (768, 4096); host sums the 8 partials, adds
proj_b, transposes.

SPMD trick: even cores' units form the pattern [(j0,half0),(j0,half1),
(j1,half0)]; odd cores' form [(j0,half1),(j1,half0),(j1,half1)]. One graph
must serve both, so odd cores get their TOKEN ORDER half-swapped on the host
(xT columns, rel_h gather built with the swapped coords, output columns
un-swapped on host). In swapped space every core sees the canonical pattern
[(0,0),(0,1),(1,0)] with local head 0 = the fully-owned head.

Device math per local head j:
  Q' (128, 4096): [scale*q^T ; RelW^T] (j=0) / [RelW^T ; scale*q^T] (j=1)
  K' (128, 4096): [k^T ; onehot(kw)]  (j=0) / [onehot(kw) ; k^T]   (j=1)
  S^T[k,q] = K'.T @ Q'  (+ PSUM-accumulated onehot(kh).T @ RelH^T)
           = scale*q.k + rel_w[q,kw] + rel_h[q,kh]
  E^T = exp(S^T) on ScalarE straight out of PSUM (|S| < ~3: no max needed)
  ctx'^T = [V|1]^T-style: lhsT = V'[k,0:64]=v, V'[k,64]=1 -> row 64 = denom
  ctx^T = ctx'^T[0:64] * (1/denom)  (partition-broadcast via K=1 matmul)
  outT += P_h^T @ ctx^T
"""

import numpy as np
import ml_dtypes

NH, HD, Hh, Ww, DIM = 12, 64, 64, 64, 768
HW = Hh * Ww  # 4096
SCALE = HD ** -0.5
NCORES = 8
BF16 = ml_dtypes.bfloat16

LAST_EXEC_NS = None
_PROGRAM = None


def _core_units(c):
    units = [(u // 2, u % 2) for u in range(3 * c, 3 * c + 3)]
    heads = sorted({h for h, _ in units})
    return units, heads


def _prep_core_inputs(c, x, qkv_w, qkv_b, proj_w, rel_pos_h, rel_pos_w):
    f32 = np.float32
    units, heads = _core_units(c)
    swapped = (c % 2 == 1)
    if swapped:
        # local head 0 must be the fully-owned head = heads[1]
        h0, h1 = heads[1], heads[0]
    else:
        h0, h1 = heads[0], heads[1]

    xflat = x.reshape(HW, DIM).astype(f32)
    if swapped:
        xflat = np.concatenate([xflat[2048:], xflat[:2048]], axis=0)
    xT = np.ascontiguousarray(xflat.T).astype(BF16)  # (768, 4096)

    def wslice(base, h):
        return qkv_w[base + h * 64: base + h * 64 + 64, :].astype(f32)

    def pack_chunks(wa, wb):  # (64,768) x2 -> (6, 128, 128) [chunk, ic, cols]
        wt = np.concatenate([wa.T, wb.T], axis=1)  # (768, 128)
        return np.ascontiguousarray(
            wt.reshape(6, 128, 128)).astype(BF16)

    wk = pack_chunks(wslice(768, h0), wslice(768, h1))
    wq = pack_chunks(SCALE * wslice(0, h0), SCALE * wslice(0, h1))
    wv = pack_chunks(wslice(1536, h0), wslice(1536, h1))
    # one partition-major tensor: wall[p, (i*3+kind)*128 : +128]
    wall = np.zeros((128, 18 * 128), dtype=BF16)
    for i in range(6):
        for kind, w in enumerate((wk, wq, wv)):
            wall[:, (i * 3 + kind) * 128:(i * 3 + kind) * 128 + 128] = w[i]

    def bvec(base):
        return np.concatenate([
            qkv_b[base + h0 * 64: base + h0 * 64 + 64],
            qkv_b[base + h1 * 64: base + h1 * 64 + 64],
        ]).astype(f32).reshape(128, 1)

    # k-bias is dropped entirely: softmax is invariant to the per-row
    # constant scale*q.kb it adds to scores, and k appears nowhere else.
    # v-bias is applied after normalization (ctx = AV/denom + vb, exact).
    qb2 = np.ascontiguousarray(SCALE * bvec(0))
    vb = bvec(1536)  # (128,1): h0 rows 0:64, h1 rows 64:128
    vbcol = np.zeros((128, 2), dtype=f32)
    vbcol[0:64, 0] = vb[0:64, 0]
    vbcol[0:64, 1] = vb[64:128, 0]
    ball = np.ascontiguousarray(
        np.concatenate([qb2, vbcol], axis=1))  # (128, 3) f32

    # rel gathers in (possibly swapped) coordinates. The h-coordinate of
    # token t_new is perm(t_new // 64) where perm(a) = (a+32)%64 for odd
    # cores; the w-coordinate is unchanged.  rel value needs ORIGINAL coords.
    a = np.arange(64)
    perm = ((a + 32) % 64) if swapped else a
    idx_h = perm[:, None] - perm[None, :] + 63     # (qh_new, kh_new)
    idx_w = a[:, None] - a[None, :] + 63           # (qw, kw)

    def gather(tab, idx):
        g = np.transpose(tab[idx], (2, 0, 1)).reshape(HD, HW) * 8.0
        return np.ascontiguousarray(
            np.concatenate([g, g], axis=0)).astype(BF16)  # (128, 4096)

    relh = gather(rel_pos_h, idx_h)
    relw = gather(rel_pos_w, idx_w)

    k = np.arange(HW)
    ohkw = np.ascontiguousarray(
        (k[None, :] % 64 == a[:, None])).astype(BF16)   # (64, 4096)
    ohkh = np.ascontiguousarray(
        (k[None, :] // 64 == a[:, None])).astype(BF16)  # (64, 4096)

    projT = np.ascontiguousarray(np.concatenate(
        [proj_w[:, h * 64: h * 64 + 64].T.astype(f32) for h in (h0, h1)],
        axis=1)).astype(BF16)  # (64, 1536)

    # rel tables + kh-onehot packed as one partition-major DMA (128, 3*HW)
    ohkh2 = np.concatenate([ohkh, ohkh], axis=0)  # (128, HW)
    relpack = np.ascontiguousarray(
        np.concatenate([relh, relw, ohkh2], axis=1))  # (128, 3*HW)

    return dict(xT=xT, wall=wall, ball=ball, relpack=relpack,
                ohkw=ohkw, projT=projT)


def _build_program():
    import concourse.bacc as bacc
    import concourse.tile as tile
    import concourse.mybir as mybir

    f32 = mybir.dt.float32
    bf16 = mybir.dt.bfloat16
    AF = mybir.ActivationFunctionType
    ALU = mybir.AluOpType

    nc = bacc.Bacc("TRN2", target_bir_lowering=False, debug=False,
                   enable_asserts=False, num_devices=NCORES)

    def din(name, shape, dt=bf16):
        return nc.dram_tensor(name, list(shape), dt, kind="ExternalInput").ap()

    xT_d = din("xT", (DIM, HW))
    wall_d = din("wall", (128, 18 * 128))
    ball_d = din("ball", (128, 3), f32)
    relpack_d = din("relpack", (128, 3 * HW))
    ohkw_d = din("ohkw", (64, HW))
    projT_d = din("projT", (64, 2 * DIM))
    outT_d = nc.dram_tensor("outT", [DIM, HW], bf16,
                            kind="ExternalOutput").ap()

    sched = [(0, 0), (0, 1), (1, 0)]  # canonical (local head j, q-half)

    with tile.TileContext(nc) as tc:
        with tc.tile_pool(name="persist", bufs=1) as P:
            # ---- constants / weights: ONE packed DMA for all qkv weights
            # and one for biases (per-DMA queue overhead is ~650 ns), then
            # the xT stream, then the big rel-table pack (needed later) ----
            wall_s = P.tile([128, 18 * 128], bf16, name="wall")
            nc.sync.dma_start(wall_s[:, :], wall_d)
            wk_s = [wall_s[:, (i * 3 + 0) * 128:(i * 3 + 0) * 128 + 128]
                    for i in range(6)]
            wq_s = [wall_s[:, (i * 3 + 1) * 128:(i * 3 + 1) * 128 + 128]
                    for i in range(6)]
            wv_s = [wall_s[:, (i * 3 + 2) * 128:(i * 3 + 2) * 128 + 128]
                    for i in range(6)]

            ball_s = P.tile([128, 3], f32, name="ball")
            nc.sync.dma_start(ball_s[:, :], ball_d)
            qb_s = ball_s[:, 0:1]
            vb_s = ball_s[:, 1:3]

            xT = [P.tile([128, HW], bf16, name=f"xT{i}") for i in range(6)]
            xT_r = xT_d.rearrange("(c p) t -> c p t", p=128)
            for i in range(6):
                nc.sync.dma_start(xT[i][:, :], xT_r[i])

            relpack_s = P.tile([128, 3 * HW], bf16, name="relpack")
            nc.sync.dma_start(relpack_s[:, :], relpack_d)
            relh_s = relpack_s[:, 0:HW]
            relw_s = relpack_s[:, HW:2 * HW]
            ohkh_s = relpack_s[:, 2 * HW:3 * HW]

            projT_s = P.tile([64, 2 * DIM], bf16, name="projT")
            nc.sync.dma_start(projT_s[:, :], projT_d)

            # ---- persistent computed tensors ----
            Kp = [P.tile([128, HW], bf16, name=f"Kp{j}") for j in range(2)]
            Qp = [P.tile([128, HW], bf16, name=f"Qp{j}") for j in range(2)]
            RelHT = [P.tile([128, HW], bf16, name=f"RelHT{j}") for j in range(2)]
            Vp = [P.tile([128, 65 * 32], bf16, name=f"Vp{j}") for j in range(2)]
            ctx_sb = [P.tile([64, 2048], bf16, name=f"ctx{u}") for u in range(3)]

            ones_s = P.tile([65, 64], f32, name="ones")
            nc.vector.memset(ones_s[64:65, :], 1.0)

            # onehot(kw) halves of K'
            nc.sync.dma_start(Kp[0][64:128, :], ohkw_d)
            nc.sync.dma_start(Kp[1][0:64, :], ohkw_d)

            # ---------------- phase 1: projections ----------------
            # K then Q, i-outer across 8 psum banks so PE starts as soon as
            # xT chunk 0 lands instead of waiting for the full 6 MB.
            with tc.tile_pool(name="p1", bufs=1, space="PSUM") as PP:
                for kind, w_s, dstA, dstB in (
                        ("k", wk_s, Kp[0], Kp[1]),
                        ("q", wq_s, Qp[0], Qp[1])):
                    pk = [PP.tile([128, 512], f32, name=f"p{kind}{t}",
                                  tag=f"pqk{t}") for t in range(8)]
                    for i in range(6):
                        for t in range(8):
                            ts = slice(t * 512, t * 512 + 512)
                            nc.tensor.matmul(pk[t][:, :], w_s[i][:, :],
                                             xT[i][:, ts],
                                             start=(i == 0), stop=(i == 5))
                    for t in range(8):
                        ts = slice(t * 512, t * 512 + 512)
                        if kind == "k":
                            nc.scalar.activation(dstA[0:64, ts],
                                                 pk[t][0:64, :], AF.Copy)
                            nc.scalar.activation(dstB[64:128, ts],
                                                 pk[t][64:128, :], AF.Copy)
                        else:
                            nc.vector.tensor_scalar_add(
                                dstA[0:64, ts], pk[t][0:64, :], qb_s[0:64, :])
                            if t < 4:  # local head 1 only serves q-half 0
                                nc.vector.tensor_scalar_add(
                                    dstB[64:128, ts], pk[t][64:128, :],
                                    qb_s[64:128, :])
                # V in [token, d] layout + ones column at d=64
                for j in range(2):
                    vp_r = Vp[j].rearrange("p (b e) -> p b e", e=65)
                    nc.vector.memset(vp_r[:, :, 64:65], 1.0)
                for b in range(32):
                    bs = slice(b * 128, b * 128 + 128)
                    pv = PP.tile([128, 128], f32, tag=f"pqk{b % 4}", name="pv")
                    for i in range(6):
                        nc.tensor.matmul(pv[:, :], xT[i][:, bs], wv_s[i][:, :],
                                         start=(i == 0), stop=(i == 5))
                    for j in range(2):
                        if (b + j) % 2 == 0:
                            nc.scalar.activation(
                                Vp[j][:, b * 65: b * 65 + 64],
                                pv[:, j * 64: j * 64 + 64], AF.Copy)
                        else:
                            nc.vector.tensor_copy(
                                Vp[j][:, b * 65: b * 65 + 64],
                                pv[:, j * 64: j * 64 + 64])

                # RelH^T[kh, q] per local head, duplicated on both partition
                # halves (so the two rel-bias matmuls of a q-tile pair can run
                # on disjoint PE row groups)
                for j in range(2):
                    rows = slice(0, 64) if j == 0 else slice(64, 128)
                    for g in range(8 if j == 0 else 4):
                        pr = PP.tile([128, 512], f32, tag=f"pqk{4 + g % 2}",
                                     name="prh")
                        for qi in range(8):
                            qh = g * 8 + qi
                            qs = slice(qh * 64, qh * 64 + 64)
                            cs = slice(qi * 64, qi * 64 + 64)
                            nc.tensor.matmul(pr[0:64, cs], relh_s[rows, qs],
                                             Qp[j][rows, qs],
                                             start=True, stop=True)
                            nc.tensor.matmul(pr[64:128, cs], relh_s[rows, qs],
                                             Qp[j][rows, qs],
                                             start=True, stop=True)
                        nc.scalar.activation(
                            RelHT[j][:, g * 512: g * 512 + 512], pr[:, :],
                            AF.Copy)

                # RelW^T into the other half of Q' (strided q columns)
                for j in range(2):
                    rows = slice(0, 64) if j == 0 else slice(64, 128)
                    orows = slice(64, 128) if j == 0 else slice(0, 64)
                    nqh = 64 if j == 0 else 32
                    qp_r = Qp[j].rearrange("p (qh qw) -> p qw qh", qw=64)
                    for g in range(8):
                        pr = PP.tile([128, 512], f32, tag=f"pqk{6 + g % 2}",
                                     name="prw")
                        for qi in range(8):
                            qw = g * 8 + qi
                            nc.tensor.matmul(
                                pr[orows, qi * nqh: qi * nqh + nqh],
                                relw_s[rows, qw * 64: qw * 64 + 64],
                                qp_r[rows, qw, 0:nqh],
                                start=True, stop=True)
                        src = pr[orows, 0:8 * nqh].rearrange(
                            "p (qw qh) -> p qw qh", qh=nqh)
                        dst = qp_r[orows, g * 8:(g + 1) * 8, 0:nqh]
                        nc.scalar.activation(dst, src, AF.Copy)

            # ---------------- phase 2 + 3 interleaved ----------------
            # Per kb: mm1(s0), mm1(s1) [shared ldweights], then the two
            # rel-bias matmuls on DISJOINT PE row groups (lo/hi copies of
            # ohkh/RelHT) so real HW runs them concurrently, one exp over
            # both banks, then the two AV matmuls [shared ldweights].
            # proj(half=1) is emitted right after unit 1 so its DVE/DMA tail
            # overlaps unit 2's compute.
            with tc.tile_pool(name="ps", bufs=3, space="PSUM") as PS, \
                 tc.tile_pool(name="pc", bufs=2, space="PSUM") as PC, \
                 tc.tile_pool(name="esb", bufs=3) as ES, \
                 tc.tile_pool(name="pos", bufs=6) as POS:

                def attention_unit(uidx, j, half, qts=(0, 1)):
                    q0 = half * 2048
                    for qt in qts:
                        qtb = q0 + qt * 1024
                        ctx_ps = [PC.tile([128, 512], f32, tag="ctx",
                                          name=f"ctxps{_s}")
                                  for _s in range(2)]
                        pend = []
                        for kb in range(32):
                            kbs = slice(kb * 128, kb * 128 + 128)
                            ps_t = PS.tile([128, 1024], f32, tag="s")
                            for s in range(2):
                                qs = slice(qtb + s * 512, qtb + s * 512 + 512)
                                nc.tensor.matmul(
                                    ps_t[:, s * 512: s * 512 + 512],
                                    Kp[j][:, kbs], Qp[j][:, qs],
                                    start=True, stop=False)
                            for s in range(2):
                                qs = slice(qtb + s * 512, qtb + s * 512 + 512)
                                rr = slice(0, 64) if s == 0 else slice(64, 128)
                                nc.tensor.matmul(
                                    ps_t[:, s * 512: s * 512 + 512],
                                    ohkh_s[rr, kbs], RelHT[j][rr, qs],
                                    start=False, stop=True)
                            e_t = ES.tile([128, 1024], bf16, tag="e", bufs=9)
                            nc.scalar.activation(e_t[:, :], ps_t[:, :], AF.Exp)
                            pend.append((kb, e_t))
                            if len(pend) > 6:
                                pkb, pe_t = pend.pop(0)
                                for s in range(2):
                                    nc.tensor.matmul(
                                        ctx_ps[s][0:65, :],
                                        Vp[j][:, pkb * 65: pkb * 65 + 65],
                                        pe_t[:, s * 512: s * 512 + 512],
                                        start=(pkb == 0), stop=False)
                        for n, (pkb, pe_t) in enumerate(pend):
                            for s in range(2):
                                nc.tensor.matmul(
                                    ctx_ps[s][0:65, :],
                                    Vp[j][:, pkb * 65: pkb * 65 + 65],
                                    pe_t[:, s * 512: s * 512 + 512],
                                    start=(pkb == 0), stop=(pkb == 31))
                        for s in range(2):
                            den = ES.tile([65, 512], f32, tag="den")
                            nc.vector.reciprocal(den[64:65, :],
                                                 ctx_ps[s][64:65, :])
                            pb_t = ctx_ps[s][64:128, :]
                            nc.tensor.matmul(pb_t, ones_s[64:65, :],
                                             den[64:65, :], start=True,
                                             stop=True,
                                             tile_position=(64, 64))
                            bc_s = ES.tile([64, 512], f32, tag="bcs")
                            nc.vector.tensor_copy(bc_s[:, 0:256],
                                                  pb_t[:, 0:256])
                            nc.scalar.activation(bc_s[:, 256:512],
                                                 pb_t[:, 256:512], AF.Copy)
                            cdst = ctx_sb[uidx][:, qt * 1024 + s * 512:
                                               qt * 1024 + s * 512 + 512]
                            tmp = ES.tile([64, 512], f32, tag="tmp")
                            nc.vector.tensor_tensor(tmp[:, :],
                                                    ctx_ps[s][0:64, :],
                                                    bc_s[:, :], op=ALU.mult)
                            nc.vector.tensor_scalar_add(
                                cdst, tmp[:, :], vb_s[0:64, j: j + 1])

                def proj(half, tail, subs=(0, 1, 2, 3)):
                    us = [(ui, j) for ui, (j, hf) in enumerate(sched)
                          if hf == half]
                    po_f = [None]
                    for n, (ocb, s) in enumerate(
                            (o, q) for o in range(6) for q in subs):
                        if tail:
                            if n % 2 == 0:
                                po_f[0] = PS.tile([128, 1024], f32, tag="s",
                                                  name="pot")
                            po_t = po_f[0][:, (n % 2) * 512:
                                           (n % 2) * 512 + 512]
                        else:
                            po_t = PC.tile([128, 512], f32, tag="ctx",
                                           name="pot")
                        for i, (ui, j) in enumerate(us):
                            nc.tensor.matmul(
                                po_t[:, :],
                                projT_s[:, j * DIM + ocb * 128:
                                        j * DIM + ocb * 128 + 128],
                                ctx_sb[ui][:, s * 512: s * 512 + 512],
                                start=(i == 0), stop=(i == len(us) - 1))
                        po_s = POS.tile([128, 512], bf16, tag="pos")
                        if tail and (ocb + s) % 2 == 0:
                            nc.scalar.activation(po_s[:, :], po_t[:, :],
                                                 AF.Copy)
                        else:
                            nc.vector.tensor_copy(po_s[:, :], po_t[:, :])
                        nc.sync.dma_start(
                            outT_d[ocb * 128: ocb * 128 + 128,
                                   half * 2048 + s * 512:
                                   half * 2048 + s * 512 + 512],
                            po_s[:, :])

                attention_unit(0, *sched[0])
                attention_unit(1, *sched[1], qts=(0,))
                proj(1, tail=False, subs=(0, 1))
                attention_unit(1, *sched[1], qts=(1,))
                proj(1, tail=False, subs=(2, 3))
                attention_unit(2, *sched[2], qts=(0,))
                proj(0, tail=False, subs=(0, 1))
                attention_unit(2, *sched[2], qts=(1,))

            with tc.tile_pool(name="potail", bufs=2, space="PSUM") as PT, \
                 tc.tile_pool(name="postail", bufs=8) as PX:
                us = [(ui, j) for ui, (j, hf) in enumerate(sched) if hf == 0]
                for ocb in range(6):
                    # both remaining q-subtiles per weight slice: one
                    # LDWEIGHTS serves 2 matmuls on real hardware
                    po_t = [PT.tile([128, 512], f32, tag=f"po{_q}",
                                    name=f"pot{_q}") for _q in range(2)]
                    for i, (ui, j) in enumerate(us):
                        for qi, sq in enumerate((2, 3)):
                            nc.tensor.matmul(
                                po_t[qi][:, :],
                                projT_s[:, j * DIM + ocb * 128:
                                        j * DIM + ocb * 128 + 128],
                                ctx_sb[ui][:, sq * 512: sq * 512 + 512],
                                start=(i == 0), stop=(i == len(us) - 1))
                    for qi, sq in enumerate((2, 3)):
                        po_s = PX.tile([128, 512], bf16, tag="pos")
                        if (ocb + sq) % 2 == 0:
                            nc.scalar.activation(po_s[:, :], po_t[qi][:, :],
                                                 AF.Copy)
                        else:
                            nc.vector.tensor_copy(po_s[:, :], po_t[qi][:, :])
                        nc.sync.dma_start(
                            outT_d[ocb * 128: ocb * 128 + 128,
                                   sq * 512: sq * 512 + 512],
                            po_s[:, :])

    nc.compile()
    return nc


def kernel(x, qkv_w, qkv_b, proj_w, proj_b, rel_pos_h, rel_pos_w, num_heads):
    global LAST_EXEC_NS, _PROGRAM
    from concourse.bass_utils import run_bass_kernel_spmd

    x = np.asarray(x, dtype=np.float32)
    qkv_w = np.asarray(qkv_w, dtype=np.float32)
    qkv_b = np.asarray(qkv_b, dtype=np.float32)
    proj_w = np.asarray(proj_w, dtype=np.float32)
    proj_b = np.asarray(proj_b, dtype=np.float32)
    rel_pos_h = np.asarray(rel_pos_h, dtype=np.float32)
    rel_pos_w = np.asarray(rel_pos_w, dtype=np.float32)
    assert int(num_heads) == NH

    in_maps = [_prep_core_inputs(c, x, qkv_w, qkv_b, proj_w,
                                 rel_pos_h, rel_pos_w) for c in range(NCORES)]

    if _PROGRAM is None:
        _PROGRAM = _build_program()
    nc = _PROGRAM

    import os
    trace = os.environ.get("KERNEL_TRACE", "0") == "1"
    res = run_bass_kernel_spmd(nc, in_maps, core_ids=list(range(NCORES)),
                               trace=trace)
    LAST_EXEC_NS = res.exec_time_ns

    out = np.zeros((DIM, HW), dtype=np.float32)
    for c in range(NCORES):
        o = np.asarray(res.results[c]["outT"], dtype=np.float32)
        if c % 2 == 1:  # un-swap token halves
            o = np.concatenate([o[:, 2048:], o[:, :2048]], axis=1)
        out += o
    out = out.T + proj_b[None, :]
    return out.reshape(1, Hh, Ww, DIM).astype(np.float32)


# revision 44
# speedup vs baseline: 1.0227x; 1.0227x over previous
"""SAM-style global attention (1,64,64,768), 12 heads, on 8 TRN2 NeuronCores.

Sharding: 24 units of (head, query-half-of-2048). Core c owns units
[3c, 3c+3) = 1.5 heads of queries spanning exactly 2 heads. Each core emits a
partial projected output outT (768, 4096); host sums the 8 partials, adds
proj_b, transposes.

SPMD trick: even cores' units form the pattern [(j0,half0),(j0,half1),
(j1,half0)]; odd cores' form [(j0,half1),(j1,half0),(j1,half1)]. One graph
must serve both, so odd cores get their TOKEN ORDER half-swapped on the host
(xT columns, rel_h gather built with the swapped coords, output columns
un-swapped on host). In swapped space every core sees the canonical pattern
[(0,0),(0,1),(1,0)] with local head 0 = the fully-owned head.

Device math per local head j:
  Q' (128, 4096): [scale*q^T ; RelW^T] (j=0) / [RelW^T ; scale*q^T] (j=1)
  K' (128, 4096): [k^T ; onehot(kw)]  (j=0) / [onehot(kw) ; k^T]   (j=1)
  S^T[k,q] = K'.T @ Q'  (+ PSUM-accumulated onehot(kh).T @ RelH^T)
           = scale*q.k + rel_w[q,kw] + rel_h[q,kh]
  E^T = exp(S^T) on ScalarE straight out of PSUM (|S| < ~3: no max needed)
  ctx'^T = [V|1]^T-style: lhsT = V'[k,0:64]=v, V'[k,64]=1 -> row 64 = denom
  ctx^T = ctx'^T[0:64] * (1/denom)  (partition-broadcast via K=1 matmul)
  outT += P_h^T @ ctx^T
"""

import numpy as np
import ml_dtypes

NH, HD, Hh, Ww, DIM = 12, 64, 64, 64, 768
HW = Hh * Ww  # 4096
SCALE = HD ** -0.5
NCORES = 8
BF16 = ml_dtypes.bfloat16

LAST_EXEC_NS = None
_PROGRAM = None


def _core_units(c):
    units = [(u // 2, u % 2) for u in range(3 * c, 3 * c + 3)]
    heads = sorted({h for h, _ in units})
    return units, heads


def _prep_core_inputs(c, x, qkv_w, qkv_b, proj_w, rel_pos_h, rel_pos_w):
    f32 = np.float32
    units, heads = _core_units(c)
    swapped = (c % 2 == 1)
    if swapped:
        # local head 0 must be the fully-owned head = heads[1]
        h0, h1 = heads[1], heads[0]
    else:
        h0, h1 = heads[0], heads[1]

    xflat = x.reshape(HW, DIM).astype(f32)
    if swapped:
        xflat = np.concatenate([xflat[2048:], xflat[:2048]], axis=0)
    xT = np.ascontiguousarray(xflat.T).astype(BF16)  # (768, 4096)

    def wslice(base, h):
        return qkv_w[base + h * 64: base + h * 64 + 64, :].astype(f32)

    def pack_chunks(wa, wb):  # (64,768) x2 -> (6, 128, 128) [chunk, ic, cols]
        wt = np.concatenate([wa.T, wb.T], axis=1)  # (768, 128)
        return np.ascontiguousarray(
            wt.reshape(6, 128, 128)).astype(BF16)

    wk = pack_chunks(wslice(768, h0), wslice(768, h1))
    wq = pack_chunks(SCALE * wslice(0, h0), SCALE * wslice(0, h1))
    wv = pack_chunks(wslice(1536, h0), wslice(1536, h1))
    # one partition-major tensor, grouped by kind so the K-weights third
    # can ride its own early DMA: wall[p, kind*768 + i*128 : +128]
    wall = np.zeros((128, 18 * 128), dtype=BF16)
    for i in range(6):
        for kind, w in enumerate((wk, wq, wv)):
            wall[:, kind * 768 + i * 128: kind * 768 + i * 128 + 128] = w[i]

    def bvec(base):
        return np.concatenate([
            qkv_b[base + h0 * 64: base + h0 * 64 + 64],
            qkv_b[base + h1 * 64: base + h1 * 64 + 64],
        ]).astype(f32).reshape(128, 1)

    # k-bias is dropped entirely: softmax is invariant to the per-row
    # constant scale*q.kb it adds to scores, and k appears nowhere else.
    # v-bias is applied after normalization (ctx = AV/denom + vb, exact).
    qb2 = np.ascontiguousarray(SCALE * bvec(0))
    vb = bvec(1536)  # (128,1): h0 rows 0:64, h1 rows 64:128
    vbcol = np.zeros((128, 2), dtype=f32)
    vbcol[0:64, 0] = vb[0:64, 0]
    vbcol[0:64, 1] = vb[64:128, 0]
    ball = np.ascontiguousarray(
        np.concatenate([qb2, vbcol], axis=1))  # (128, 3) f32

    # rel gathers in (possibly swapped) coordinates. The h-coordinate of
    # token t_new is perm(t_new // 64) where perm(a) = (a+32)%64 for odd
    # cores; the w-coordinate is unchanged.  rel value needs ORIGINAL coords.
    a = np.arange(64)
    perm = ((a + 32) % 64) if swapped else a
    idx_h = perm[:, None] - perm[None, :] + 63     # (qh_new, kh_new)
    idx_w = a[:, None] - a[None, :] + 63           # (qw, kw)

    def gather(tab, idx):
        g = np.transpose(tab[idx], (2, 0, 1)).reshape(HD, HW) * 8.0
        return np.ascontiguousarray(
            np.concatenate([g, g], axis=0)).astype(BF16)  # (128, 4096)

    relh = gather(rel_pos_h, idx_h)
    relw = gather(rel_pos_w, idx_w)

    k = np.arange(HW)
    ohkw = np.ascontiguousarray(
        (k[None, :] % 64 == a[:, None])).astype(BF16)   # (64, 4096)
    ohkh = np.ascontiguousarray(
        (k[None, :] // 64 == a[:, None])).astype(BF16)  # (64, 4096)

    projT = np.ascontiguousarray(np.concatenate(
        [proj_w[:, h * 64: h * 64 + 64].T.astype(f32) for h in (h0, h1)],
        axis=1)).astype(BF16)  # (64, 1536)

    # rel tables + kh-onehot packed as one partition-major DMA (128, 3*HW)
    ohkh2 = np.concatenate([ohkh, ohkh], axis=0)  # (128, HW)
    relpack = np.ascontiguousarray(
        np.concatenate([relh, relw, ohkh2], axis=1))  # (128, 3*HW)

    return dict(xT=xT, wall=wall, ball=ball, relpack=relpack,
                ohkw=ohkw, projT=projT)


def _build_program():
    import concourse.bacc as bacc
    import concourse.tile as tile
    import concourse.mybir as mybir

    f32 = mybir.dt.float32
    bf16 = mybir.dt.bfloat16
    AF = mybir.ActivationFunctionType
    ALU = mybir.AluOpType

    nc = bacc.Bacc("TRN2", target_bir_lowering=False, debug=False,
                   enable_asserts=False, num_devices=NCORES)

    def din(name, shape, dt=bf16):
        return nc.dram_tensor(name, list(shape), dt, kind="ExternalInput").ap()

    xT_d = din("xT", (DIM, HW))
    wall_d = din("wall", (128, 18 * 128))
    ball_d = din("ball", (128, 3), f32)
    relpack_d = din("relpack", (128, 3 * HW))
    ohkw_d = din("ohkw", (64, HW))
    projT_d = din("projT", (64, 2 * DIM))
    outT_d = nc.dram_tensor("outT", [DIM, HW], bf16,
                            kind="ExternalOutput").ap()

    sched = [(0, 0), (0, 1), (1, 0)]  # canonical (local head j, q-half)

    with tile.TileContext(nc) as tc:
        with tc.tile_pool(name="persist", bufs=1) as P:
            # ---- constants / weights: ONE packed DMA for all qkv weights
            # and one for biases (per-DMA queue overhead is ~650 ns), then
            # the xT stream, then the big rel-table pack (needed later) ----
            wall_s = P.tile([128, 18 * 128], bf16, name="wall")
            nc.sync.dma_start(wall_s[:, 0:768], wall_d[:, 0:768])
            wk_s = [wall_s[:, i * 128: i * 128 + 128] for i in range(6)]
            wq_s = [wall_s[:, 768 + i * 128: 768 + i * 128 + 128]
                    for i in range(6)]
            wv_s = [wall_s[:, 1536 + i * 128: 1536 + i * 128 + 128]
                    for i in range(6)]

            xT = [P.tile([128, HW], bf16, name=f"xT{i}") for i in range(6)]
            xT_r = xT_d.rearrange("(c p) t -> c p t", p=128)
            for i in range(6):
                nc.sync.dma_start(xT[i][:, :], xT_r[i])

            # wq/wv (needed ~20us in) and biases (later) ride after xT
            nc.sync.dma_start(wall_s[:, 768:2304], wall_d[:, 768:2304])
            ball_s = P.tile([128, 3], f32, name="ball")
            nc.sync.dma_start(ball_s[:, :], ball_d)
            qb_s = ball_s[:, 0:1]
            vb_s = ball_s[:, 1:3]

            relpack_s = P.tile([128, 3 * HW], bf16, name="relpack")
            nc.sync.dma_start(relpack_s[:, :], relpack_d)
            relh_s = relpack_s[:, 0:HW]
            relw_s = relpack_s[:, HW:2 * HW]
            ohkh_s = relpack_s[:, 2 * HW:3 * HW]

            projT_s = P.tile([64, 2 * DIM], bf16, name="projT")
            nc.sync.dma_start(projT_s[:, :], projT_d)

            # ---- persistent computed tensors ----
            Kp = [P.tile([128, HW], bf16, name=f"Kp{j}") for j in range(2)]
            Qp = [P.tile([128, HW], bf16, name=f"Qp{j}") for j in range(2)]
            RelHT = [P.tile([128, HW], bf16, name=f"RelHT{j}") for j in range(2)]
            Vp = [P.tile([128, 65 * 32], bf16, name=f"Vp{j}") for j in range(2)]
            ctx_sb = [P.tile([64, 2048], bf16, name=f"ctx{u}") for u in range(3)]

            ones_s = P.tile([65, 64], f32, name="ones")
            nc.vector.memset(ones_s[64:65, :], 1.0)

            # onehot(kw) halves of K'
            nc.sync.dma_start(Kp[0][64:128, :], ohkw_d)
            nc.sync.dma_start(Kp[1][0:64, :], ohkw_d)

            # ---------------- phase 1: projections ----------------
            # K then Q, i-outer across 8 psum banks so PE starts as soon as
            # xT chunk 0 lands instead of waiting for the full 6 MB.
            with tc.tile_pool(name="p1", bufs=1, space="PSUM") as PP:
                for kind, w_s, dstA, dstB in (
                        ("k", wk_s, Kp[0], Kp[1]),
                        ("q", wq_s, Qp[0], Qp[1])):
                    pk = [PP.tile([128, 512], f32, name=f"p{kind}{t}",
                                  tag=f"pqk{t}") for t in range(8)]
                    for i in range(6):
                        for t in range(8):
                            ts = slice(t * 512, t * 512 + 512)
                            nc.tensor.matmul(pk[t][:, :], w_s[i][:, :],
                                             xT[i][:, ts],
                                             start=(i == 0), stop=(i == 5))
                    for t in range(8):
                        ts = slice(t * 512, t * 512 + 512)
                        if kind == "k":
                            nc.scalar.activation(dstA[0:64, ts],
                                                 pk[t][0:64, :], AF.Copy)
                            nc.scalar.activation(dstB[64:128, ts],
                                                 pk[t][64:128, :], AF.Copy)
                        else:
                            nc.vector.tensor_scalar_add(
                                dstA[0:64, ts], pk[t][0:64, :], qb_s[0:64, :])
                            if t < 4:  # local head 1 only serves q-half 0
                                nc.vector.tensor_scalar_add(
                                    dstB[64:128, ts], pk[t][64:128, :],
                                    qb_s[64:128, :])
                # V in [token, d] layout + ones column at d=64
                for j in range(2):
                    vp_r = Vp[j].rearrange("p (b e) -> p b e", e=65)
                    nc.vector.memset(vp_r[:, :, 64:65], 1.0)
                for b in range(32):
                    bs = slice(b * 128, b * 128 + 128)
                    pv = PP.tile([128, 128], f32, tag=f"pqk{b % 4}", name="pv")
                    for i in range(6):
                        nc.tensor.matmul(pv[:, :], xT[i][:, bs], wv_s[i][:, :],
                                         start=(i == 0), stop=(i == 5))
                    for j in range(2):
                        if (b + j) % 2 == 0:
                            nc.scalar.activation(
                                Vp[j][:, b * 65: b * 65 + 64],
                                pv[:, j * 64: j * 64 + 64], AF.Copy)
                        else:
                            nc.vector.tensor_copy(
                                Vp[j][:, b * 65: b * 65 + 64],
                                pv[:, j * 64: j * 64 + 64])

                # RelH^T[kh, q] per local head, duplicated on both partition
                # halves (so the two rel-bias matmuls of a q-tile pair can run
                # on disjoint PE row groups)
                for j in range(2):
                    rows = slice(0, 64) if j == 0 else slice(64, 128)
                    for g in range(8 if j == 0 else 4):
                        pr = PP.tile([128, 512], f32, tag=f"pqk{4 + g % 2}",
                                     name="prh")
                        for qi in range(8):
                            qh = g * 8 + qi
                            qs = slice(qh * 64, qh * 64 + 64)
                            cs = slice(qi * 64, qi * 64 + 64)
                            nc.tensor.matmul(pr[0:64, cs], relh_s[rows, qs],
                                             Qp[j][rows, qs],
                                             start=True, stop=True)
                            nc.tensor.matmul(pr[64:128, cs], relh_s[rows, qs],
                                             Qp[j][rows, qs],
                                             start=True, stop=True)
                        if g % 2 == 0:
                            nc.scalar.activation(
                                RelHT[j][:, g * 512: g * 512 + 512],
                                pr[:, :], AF.Copy)
                        else:
                            nc.vector.tensor_copy(
                                RelHT[j][:, g * 512: g * 512 + 512],
                                pr[:, :])

                # RelW^T into the other half of Q' (strided q columns)
                for j in range(2):
                    rows = slice(0, 64) if j == 0 else slice(64, 128)
                    orows = slice(64, 128) if j == 0 else slice(0, 64)
                    nqh = 64 if j == 0 else 32
                    qp_r = Qp[j].rearrange("p (qh qw) -> p qw qh", qw=64)
                    for g in range(8):
                        pr = PP.tile([128, 512], f32, tag=f"pqk{6 + g % 2}",
                                     name="prw")
                        for qi in range(8):
                            qw = g * 8 + qi
                            nc.tensor.matmul(
                                pr[orows, qi * nqh: qi * nqh + nqh],
                                relw_s[rows, qw * 64: qw * 64 + 64],
                                qp_r[rows, qw, 0:nqh],
                                start=True, stop=True)
                        src = pr[orows, 0:8 * nqh].rearrange(
                            "p (qw qh) -> p qw qh", qh=nqh)
                        dst = qp_r[orows, g * 8:(g + 1) * 8, 0:nqh]
                        if g % 2 == 0:
                            nc.scalar.activation(dst, src, AF.Copy)
                        else:
                            nc.vector.tensor_copy(dst, src)

            # ---------------- phase 2 + 3 interleaved ----------------
            # Per kb: mm1(s0), mm1(s1) [shared ldweights], then the two
            # rel-bias matmuls on DISJOINT PE row groups (lo/hi copies of
            # ohkh/RelHT) so real HW runs them concurrently, one exp over
            # both banks, then the two AV matmuls [shared ldweights].
            # proj(half=1) is emitted right after unit 1 so its DVE/DMA tail
            # overlaps unit 2's compute.
            with tc.tile_pool(name="ps", bufs=3, space="PSUM") as PS, \
                 tc.tile_pool(name="pc", bufs=2, space="PSUM") as PC, \
                 tc.tile_pool(name="esb", bufs=3) as ES, \
                 tc.tile_pool(name="pos", bufs=6) as POS:

                def attention_unit(uidx, j, half, qts=(0, 1)):
                    q0 = half * 2048
                    for qt in qts:
                        qtb = q0 + qt * 1024
                        ctx_ps = [PC.tile([128, 512], f32, tag="ctx",
                                          name=f"ctxps{_s}")
                                  for _s in range(2)]
                        pend = []
                        for kb in range(32):
                            kbs = slice(kb * 128, kb * 128 + 128)
                            ps_t = PS.tile([128, 1024], f32, tag="s")
                            for s in range(2):
                                qs = slice(qtb + s * 512, qtb + s * 512 + 512)
                                nc.tensor.matmul(
                                    ps_t[:, s * 512: s * 512 + 512],
                                    Kp[j][:, kbs], Qp[j][:, qs],
                                    start=True, stop=False)
                            for s in range(2):
                                qs = slice(qtb + s * 512, qtb + s * 512 + 512)
                                rr = slice(0, 64) if s == 0 else slice(64, 128)
                                nc.tensor.matmul(
                                    ps_t[:, s * 512: s * 512 + 512],
                                    ohkh_s[rr, kbs], RelHT[j][rr, qs],
                                    start=False, stop=True)
                            e_t = ES.tile([128, 1024], bf16, tag="e", bufs=9)
                            nc.scalar.activation(e_t[:, :], ps_t[:, :], AF.Exp)
                            pend.append((kb, e_t))
                            if len(pend) > 6:
                                pkb, pe_t = pend.pop(0)
                                for s in range(2):
                                    nc.tensor.matmul(
                                        ctx_ps[s][0:65, :],
                                        Vp[j][:, pkb * 65: pkb * 65 + 65],
                                        pe_t[:, s * 512: s * 512 + 512],
                                        start=(pkb == 0), stop=False)
                        for n, (pkb, pe_t) in enumerate(pend):
                            for s in range(2):
                                nc.tensor.matmul(
                                    ctx_ps[s][0:65, :],
                                    Vp[j][:, pkb * 65: pkb * 65 + 65],
                                    pe_t[:, s * 512: s * 512 + 512],
                                    start=(pkb == 0), stop=(pkb == 31))
                        for s in range(2):
                            den = ES.tile([65, 512], f32, tag="den")
                            nc.vector.reciprocal(den[64:65, :],
                                                 ctx_ps[s][64:65, :])
                            pb_t = ctx_ps[s][64:128, :]
                            nc.tensor.matmul(pb_t, ones_s[64:65, :],
                                             den[64:65, :], start=True,
                                             stop=True,
                                             tile_position=(64, 64))
                            bc_s = ES.tile([64, 512], f32, tag="bcs")
                            nc.vector.tensor_copy(bc_s[:, 0:256],
                                                  pb_t[:, 0:256])
                            nc.scalar.activation(bc_s[:, 256:512],
                                                 pb_t[:, 256:512], AF.Copy)
                            cdst = ctx_sb[uidx][:, qt * 1024 + s * 512:
                                               qt * 1024 + s * 512 + 512]
                            tmp = ES.tile([64, 512], f32, tag="tmp")
                            nc.vector.tensor_tensor(tmp[:, :],
                                                    ctx_ps[s][0:64, :],
                                                    bc_s[:, :], op=ALU.mult)
                            nc.vector.tensor_scalar_add(
                                cdst, tmp[:, :], vb_s[0:64, j: j + 1])

                def proj(half, tail, subs=(0, 1, 2, 3)):
                    us = [(ui, j) for ui, (j, hf) in enumerate(sched)
                          if hf == half]
                    po_f = [None]
                    for n, (ocb, s) in enumerate(
                            (o, q) for o in range(6) for q in subs):
                        if tail:
                            if n % 2 == 0:
                                po_f[0] = PS.tile([128, 1024], f32, tag="s",
                                                  name="pot")
                            po_t = po_f[0][:, (n % 2) * 512:
                                           (n % 2) * 512 + 512]
                        else:
                            po_t = PC.tile([128, 512], f32, tag="ctx",
                                           name="pot")
                        for i, (ui, j) in enumerate(us):
                            nc.tensor.matmul(
                                po_t[:, :],
                                projT_s[:, j * DIM + ocb * 128:
                                        j * DIM + ocb * 128 + 128],
                                ctx_sb[ui][:, s * 512: s * 512 + 512],
                                start=(i == 0), stop=(i == len(us) - 1))
                        po_s = POS.tile([128, 512], bf16, tag="pos")
                        if tail and (ocb + s) % 2 == 0:
                            nc.scalar.activation(po_s[:, :], po_t[:, :],
                                                 AF.Copy)
                        else:
                            nc.vector.tensor_copy(po_s[:, :], po_t[:, :])
                        nc.sync.dma_start(
                            outT_d[ocb * 128: ocb * 128 + 128,
                                   half * 2048 + s * 512:
                                   half * 2048 + s * 512 + 512],
                            po_s[:, :])

                attention_unit(0, *sched[0])
                attention_unit(1, *sched[1], qts=(0,))
                proj(1, tail=False, subs=(0, 1))
                attention_unit(1, *sched[1], qts=(1,))
                proj(1, tail=False, subs=(2, 3))
                attention_unit(2, *sched[2], qts=(0,))
                proj(0, tail=False, subs=(0, 1))
                attention_unit(2, *sched[2], qts=(1,))

            with tc.tile_pool(name="potail", bufs=2, space="PSUM") as PT, \
                 tc.tile_pool(name="postail", bufs=8) as PX:
                us = [(ui, j) for ui, (j, hf) in enumerate(sched) if hf == 0]
                for ocb in range(6):
                    # both remaining q-subtiles per weight slice: one
                    # LDWEIGHTS serves 2 matmuls on real hardware
                    po_t = [PT.tile([128, 512], f32, tag=f"po{_q}",
                                    name=f"pot{_q}") for _q in range(2)]
                    for i, (ui, j) in enumerate(us):
                        for qi, sq in enumerate((2, 3)):
                            nc.tensor.matmul(
                                po_t[qi][:, :],
                                projT_s[:, j * DIM + ocb * 128:
                                        j * DIM + ocb * 128 + 128],
                                ctx_sb[ui][:, sq * 512: sq * 512 + 512],
                                start=(i == 0), stop=(i == len(us) - 1))
                    for qi, sq in enumerate((2, 3)):
                        po_s = PX.tile([128, 512], bf16, tag="pos")
                        if (ocb + sq) % 2 == 0:
                            nc.scalar.activation(po_s[:, :], po_t[qi][:, :],
                                                 AF.Copy)
                        else:
                            nc.vector.tensor_copy(po_s[:, :], po_t[qi][:, :])
                        nc.sync.dma_start(
                            outT_d[ocb * 128: ocb * 128 + 128,
                                   sq * 512: sq * 512 + 512],
                            po_s[:, :])

    nc.compile()
    return nc


def kernel(x, qkv_w, qkv_b, proj_w, proj_b, rel_pos_h, rel_pos_w, num_heads):
    global LAST_EXEC_NS, _PROGRAM
    from concourse.bass_utils import run_bass_kernel_spmd

    x = np.asarray(x, dtype=np.float32)
    qkv_w = np.asarray(qkv_w, dtype=np.float32)
    qkv_b = np.asarray(qkv_b, dtype=np.float32)
    proj_w = np.asarray(proj_w, dtype=np.float32)
    proj_b = np.asarray(proj_b, dtype=np.float32)
    rel_pos_h = np.asarray(rel_pos_h, dtype=np.float32)
    rel_pos_w = np.asarray(rel_pos_w, dtype=np.float32)
    assert int(num_heads) == NH

    in_maps = [_prep_core_inputs(c, x, qkv_w, qkv_b, proj_w,
                                 rel_pos_h, rel_pos_w) for c in range(NCORES)]

    if _PROGRAM is None:
        _PROGRAM = _build_program()
    nc = _PROGRAM

    import os
    trace = os.environ.get("KERNEL_TRACE", "0") == "1"
    res = run_bass_kernel_spmd(nc, in_maps, core_ids=list(range(NCORES)),
                               trace=trace)
    LAST_EXEC_NS = res.exec_time_ns

    out = np.zeros((DIM, HW), dtype=np.float32)
    for c in range(NCORES):
        o = np.asarray(res.results[c]["outT"], dtype=np.float32)
        if c % 2 == 1:  # un-swap token halves
            o = np.concatenate([o[:, 2048:], o[:, :2048]], axis=1)
        out += o
    out = out.T + proj_b[None, :]
    return out.reshape(1, Hh, Ww, DIM).astype(np.float32)


# revision 45
# speedup vs baseline: 1.0436x; 1.0205x over previous
"""SAM-style global attention (1,64,64,768), 12 heads, on 8 TRN2 NeuronCores.

Sharding: 24 units of (head, query-half-of-2048). Core c owns units
[3c, 3c+3) = 1.5 heads of queries spanning exactly 2 heads. Each core emits a
partial projected output outT (768, 4096); host sums the 8 partials, adds
proj_b, transposes.

SPMD trick: even cores' units form the pattern [(j0,half0),(j0,half1),
(j1,half0)]; odd cores' form [(j0,half1),(j1,half0),(j1,half1)]. One graph
must serve both, so odd cores get their TOKEN ORDER half-swapped on the host
(xT columns, rel_h gather built with the swapped coords, output columns
un-swapped on host). In swapped space every core sees the canonical pattern
[(0,0),(0,1),(1,0)] with local head 0 = the fully-owned head.

Device math per local head j:
  Q' (128, 4096): [scale*q^T ; RelW^T] (j=0) / [RelW^T ; scale*q^T] (j=1)
  K' (128, 4096): [k^T ; onehot(kw)]  (j=0) / [onehot(kw) ; k^T]   (j=1)
  S^T[k,q] = K'.T @ Q'  (+ PSUM-accumulated onehot(kh).T @ RelH^T)
           = scale*q.k + rel_w[q,kw] + rel_h[q,kh]
  E^T = exp(S^T) on ScalarE straight out of PSUM (|S| < ~3: no max needed)
  ctx'^T = [V|1]^T-style: lhsT = V'[k,0:64]=v, V'[k,64]=1 -> row 64 = denom
  ctx^T = ctx'^T[0:64] * (1/denom)  (partition-broadcast via K=1 matmul)
  outT += P_h^T @ ctx^T
"""

import numpy as np
import ml_dtypes

NH, HD, Hh, Ww, DIM = 12, 64, 64, 64, 768
HW = Hh * Ww  # 4096
SCALE = HD ** -0.5
NCORES = 8
BF16 = ml_dtypes.bfloat16

LAST_EXEC_NS = None
_PROGRAM = None


def _core_units(c):
    units = [(u // 2, u % 2) for u in range(3 * c, 3 * c + 3)]
    heads = sorted({h for h, _ in units})
    return units, heads


def _prep_core_inputs(c, x, qkv_w, qkv_b, proj_w, rel_pos_h, rel_pos_w):
    f32 = np.float32
    units, heads = _core_units(c)
    swapped = (c % 2 == 1)
    if swapped:
        # local head 0 must be the fully-owned head = heads[1]
        h0, h1 = heads[1], heads[0]
    else:
        h0, h1 = heads[0], heads[1]

    xflat = x.reshape(HW, DIM).astype(f32)
    if swapped:
        xflat = np.concatenate([xflat[2048:], xflat[:2048]], axis=0)
    xT = np.ascontiguousarray(xflat.T).astype(BF16)  # (768, 4096)

    def wslice(base, h):
        return qkv_w[base + h * 64: base + h * 64 + 64, :].astype(f32)

    def pack_chunks(wa, wb):  # (64,768) x2 -> (6, 128, 128) [chunk, ic, cols]
        wt = np.concatenate([wa.T, wb.T], axis=1)  # (768, 128)
        return np.ascontiguousarray(
            wt.reshape(6, 128, 128)).astype(BF16)

    wk = pack_chunks(wslice(768, h0), wslice(768, h1))
    wq = pack_chunks(SCALE * wslice(0, h0), SCALE * wslice(0, h1))
    wv = pack_chunks(wslice(1536, h0), wslice(1536, h1))
    # one partition-major tensor, grouped by kind so the K-weights third
    # can ride its own early DMA: wall[p, kind*768 + i*128 : +128]
    wall = np.zeros((128, 18 * 128), dtype=BF16)
    for i in range(6):
        for kind, w in enumerate((wk, wq, wv)):
            wall[:, kind * 768 + i * 128: kind * 768 + i * 128 + 128] = w[i]

    def bvec(base):
        return np.concatenate([
            qkv_b[base + h0 * 64: base + h0 * 64 + 64],
            qkv_b[base + h1 * 64: base + h1 * 64 + 64],
        ]).astype(f32).reshape(128, 1)

    # k-bias is dropped entirely: softmax is invariant to the per-row
    # constant scale*q.kb it adds to scores, and k appears nowhere else.
    # v-bias is applied after normalization (ctx = AV/denom + vb, exact).
    qb2 = np.ascontiguousarray(SCALE * bvec(0))
    vb = bvec(1536)  # (128,1): h0 rows 0:64, h1 rows 64:128
    vbcol = np.zeros((128, 2), dtype=f32)
    vbcol[0:64, 0] = vb[0:64, 0]
    vbcol[0:64, 1] = vb[64:128, 0]
    ball = np.ascontiguousarray(
        np.concatenate([qb2, vbcol], axis=1))  # (128, 3) f32

    # rel gathers in (possibly swapped) coordinates. The h-coordinate of
    # token t_new is perm(t_new // 64) where perm(a) = (a+32)%64 for odd
    # cores; the w-coordinate is unchanged.  rel value needs ORIGINAL coords.
    a = np.arange(64)
    perm = ((a + 32) % 64) if swapped else a
    idx_h = perm[:, None] - perm[None, :] + 63     # (qh_new, kh_new)
    idx_w = a[:, None] - a[None, :] + 63           # (qw, kw)

    def gather(tab, idx):
        g = np.transpose(tab[idx], (2, 0, 1)).reshape(HD, HW) * 8.0
        return np.ascontiguousarray(
            np.concatenate([g, g], axis=0)).astype(BF16)  # (128, 4096)

    relh = gather(rel_pos_h, idx_h)
    relw = gather(rel_pos_w, idx_w)

    k = np.arange(HW)
    ohkw = np.ascontiguousarray(
        (k[None, :] % 64 == a[:, None])).astype(BF16)   # (64, 4096)
    ohkh = np.ascontiguousarray(
        (k[None, :] // 64 == a[:, None])).astype(BF16)  # (64, 4096)

    projT = np.ascontiguousarray(np.concatenate(
        [proj_w[:, h * 64: h * 64 + 64].T.astype(f32) for h in (h0, h1)],
        axis=1)).astype(BF16)  # (64, 1536)

    # rel tables + kh-onehot packed as one partition-major DMA (128, 3*HW)
    ohkh2 = np.concatenate([ohkh, ohkh], axis=0)  # (128, HW)
    relpack = np.ascontiguousarray(
        np.concatenate([relh, relw, ohkh2], axis=1))  # (128, 3*HW)

    return dict(xT=xT, wall=wall, ball=ball, relpack=relpack,
                ohkw=ohkw, projT=projT)


def _build_program():
    import concourse.bacc as bacc
    import concourse.tile as tile
    import concourse.mybir as mybir

    f32 = mybir.dt.float32
    bf16 = mybir.dt.bfloat16
    AF = mybir.ActivationFunctionType
    ALU = mybir.AluOpType

    nc = bacc.Bacc("TRN2", target_bir_lowering=False, debug=False,
                   enable_asserts=False, num_devices=NCORES)

    def din(name, shape, dt=bf16):
        return nc.dram_tensor(name, list(shape), dt, kind="ExternalInput").ap()

    xT_d = din("xT", (DIM, HW))
    wall_d = din("wall", (128, 18 * 128))
    ball_d = din("ball", (128, 3), f32)
    relpack_d = din("relpack", (128, 3 * HW))
    ohkw_d = din("ohkw", (64, HW))
    projT_d = din("projT", (64, 2 * DIM))
    outT_d = nc.dram_tensor("outT", [DIM, HW], bf16,
                            kind="ExternalOutput").ap()

    sched = [(0, 0), (0, 1), (1, 0)]  # canonical (local head j, q-half)

    with tile.TileContext(nc) as tc:
        with tc.tile_pool(name="persist", bufs=1) as P:
            # ---- constants / weights: ONE packed DMA for all qkv weights
            # and one for biases (per-DMA queue overhead is ~650 ns), then
            # the xT stream, then the big rel-table pack (needed later) ----
            wall_s = P.tile([128, 18 * 128], bf16, name="wall")
            nc.sync.dma_start(wall_s[:, 0:768], wall_d[:, 0:768])
            wk_s = [wall_s[:, i * 128: i * 128 + 128] for i in range(6)]
            wq_s = [wall_s[:, 768 + i * 128: 768 + i * 128 + 128]
                    for i in range(6)]
            wv_s = [wall_s[:, 1536 + i * 128: 1536 + i * 128 + 128]
                    for i in range(6)]

            xT = [P.tile([128, HW], bf16, name=f"xT{i}") for i in range(6)]
            xT_r = xT_d.rearrange("(c p) t -> c p t", p=128)
            for i in range(6):
                nc.sync.dma_start(xT[i][:, :], xT_r[i])

            # wq/wv (needed ~20us in) and biases (later) ride after xT
            nc.sync.dma_start(wall_s[:, 768:2304], wall_d[:, 768:2304])
            ball_s = P.tile([128, 3], f32, name="ball")
            nc.sync.dma_start(ball_s[:, :], ball_d)
            qb_s = ball_s[:, 0:1]
            vb_s = ball_s[:, 1:3]

            relpack_s = P.tile([128, 3 * HW], bf16, name="relpack")
            nc.sync.dma_start(relpack_s[:, :], relpack_d)
            relh_s = relpack_s[:, 0:HW]
            relw_s = relpack_s[:, HW:2 * HW]
            ohkh_s = relpack_s[:, 2 * HW:3 * HW]

            projT_s = P.tile([64, 2 * DIM], bf16, name="projT")
            nc.sync.dma_start(projT_s[:, :], projT_d)

            # ---- persistent computed tensors ----
            Kp = [P.tile([128, HW], bf16, name=f"Kp{j}") for j in range(2)]
            Qp = [P.tile([128, HW], bf16, name=f"Qp{j}") for j in range(2)]
            RelHT = [P.tile([128, HW], bf16, name=f"RelHT{j}") for j in range(2)]
            Vp = [P.tile([128, 65 * 32], bf16, name=f"Vp{j}") for j in range(2)]
            ctx_sb = [P.tile([64, 2048], bf16, name=f"ctx{u}") for u in range(3)]

            ones_s = P.tile([65, 64], bf16, name="ones")
            nc.vector.memset(ones_s[64:65, :], 1.0)

            # onehot(kw) halves of K'
            nc.sync.dma_start(Kp[0][64:128, :], ohkw_d)
            nc.sync.dma_start(Kp[1][0:64, :], ohkw_d)

            # ---------------- phase 1: projections ----------------
            # K then Q, i-outer across 8 psum banks so PE starts as soon as
            # xT chunk 0 lands instead of waiting for the full 6 MB.
            with tc.tile_pool(name="p1", bufs=1, space="PSUM") as PP:
                for kind, w_s, dstA, dstB in (
                        ("k", wk_s, Kp[0], Kp[1]),
                        ("q", wq_s, Qp[0], Qp[1])):
                    pk = [PP.tile([128, 512], f32, name=f"p{kind}{t}",
                                  tag=f"pqk{t}") for t in range(8)]
                    for i in range(6):
                        for t in range(8):
                            ts = slice(t * 512, t * 512 + 512)
                            nc.tensor.matmul(pk[t][:, :], w_s[i][:, :],
                                             xT[i][:, ts],
                                             start=(i == 0), stop=(i == 5))
                    for t in range(8):
                        ts = slice(t * 512, t * 512 + 512)
                        if kind == "k":
                            nc.scalar.activation(dstA[0:64, ts],
                                                 pk[t][0:64, :], AF.Copy)
                            nc.scalar.activation(dstB[64:128, ts],
                                                 pk[t][64:128, :], AF.Copy)
                        else:
                            nc.vector.tensor_scalar_add(
                                dstA[0:64, ts], pk[t][0:64, :], qb_s[0:64, :])
                            if t < 4:  # local head 1 only serves q-half 0
                                nc.vector.tensor_scalar_add(
                                    dstB[64:128, ts], pk[t][64:128, :],
                                    qb_s[64:128, :])
                # V in [token, d] layout + ones column at d=64
                for j in range(2):
                    vp_r = Vp[j].rearrange("p (b e) -> p b e", e=65)
                    nc.vector.memset(vp_r[:, :, 64:65], 1.0)
                for b in range(32):
                    bs = slice(b * 128, b * 128 + 128)
                    pv = PP.tile([128, 128], f32, tag=f"pqk{b % 4}", name="pv")
                    for i in range(6):
                        nc.tensor.matmul(pv[:, :], xT[i][:, bs], wv_s[i][:, :],
                                         start=(i == 0), stop=(i == 5))
                    for j in range(2):
                        if (b + j) % 2 == 0:
                            nc.scalar.activation(
                                Vp[j][:, b * 65: b * 65 + 64],
                                pv[:, j * 64: j * 64 + 64], AF.Copy)
                        else:
                            nc.vector.tensor_copy(
                                Vp[j][:, b * 65: b * 65 + 64],
                                pv[:, j * 64: j * 64 + 64])

                # RelH^T[kh, q] per local head, duplicated on both partition
                # halves (so the two rel-bias matmuls of a q-tile pair can run
                # on disjoint PE row groups)
                for j in range(2):
                    rows = slice(0, 64) if j == 0 else slice(64, 128)
                    for g in range(8 if j == 0 else 4):
                        pr = PP.tile([128, 512], f32, tag=f"pqk{4 + g % 2}",
                                     name="prh")
                        for qi in range(8):
                            qh = g * 8 + qi
                            qs = slice(qh * 64, qh * 64 + 64)
                            cs = slice(qi * 64, qi * 64 + 64)
                            nc.tensor.matmul(pr[0:64, cs], relh_s[rows, qs],
                                             Qp[j][rows, qs],
                                             start=True, stop=True)
                            nc.tensor.matmul(pr[64:128, cs], relh_s[rows, qs],
                                             Qp[j][rows, qs],
                                             start=True, stop=True)
                        if g % 2 == 0:
                            nc.scalar.activation(
                                RelHT[j][:, g * 512: g * 512 + 512],
                                pr[:, :], AF.Copy)
                        else:
                            nc.vector.tensor_copy(
                                RelHT[j][:, g * 512: g * 512 + 512],
                                pr[:, :])

                # RelW^T into the other half of Q' (strided q columns)
                for j in range(2):
                    rows = slice(0, 64) if j == 0 else slice(64, 128)
                    orows = slice(64, 128) if j == 0 else slice(0, 64)
                    nqh = 64 if j == 0 else 32
                    qp_r = Qp[j].rearrange("p (qh qw) -> p qw qh", qw=64)
                    for g in range(8):
                        pr = PP.tile([128, 512], f32, tag=f"pqk{6 + g % 2}",
                                     name="prw")
                        for qi in range(8):
                            qw = g * 8 + qi
                            nc.tensor.matmul(
                                pr[orows, qi * nqh: qi * nqh + nqh],
                                relw_s[rows, qw * 64: qw * 64 + 64],
                                qp_r[rows, qw, 0:nqh],
                                start=True, stop=True)
                        src = pr[orows, 0:8 * nqh].rearrange(
                            "p (qw qh) -> p qw qh", qh=nqh)
                        dst = qp_r[orows, g * 8:(g + 1) * 8, 0:nqh]
                        if g % 2 == 0:
                            nc.scalar.activation(dst, src, AF.Copy)
                        else:
                            nc.vector.tensor_copy(dst, src)

            # ---------------- phase 2 + 3 interleaved ----------------
            # Per kb: mm1(s0), mm1(s1) [shared ldweights], then the two
            # rel-bias matmuls on DISJOINT PE row groups (lo/hi copies of
            # ohkh/RelHT) so real HW runs them concurrently, one exp over
            # both banks, then the two AV matmuls [shared ldweights].
            # proj(half=1) is emitted right after unit 1 so its DVE/DMA tail
            # overlaps unit 2's compute.
            with tc.tile_pool(name="ps", bufs=3, space="PSUM") as PS, \
                 tc.tile_pool(name="pc", bufs=2, space="PSUM") as PC, \
                 tc.tile_pool(name="esb", bufs=3) as ES, \
                 tc.tile_pool(name="pos", bufs=6) as POS:

                def attention_unit(uidx, j, half, qts=(0, 1)):
                    q0 = half * 2048
                    for qt in qts:
                        qtb = q0 + qt * 1024
                        ctx_ps = [PC.tile([128, 512], f32, tag="ctx",
                                          name=f"ctxps{_s}")
                                  for _s in range(2)]
                        pend = []
                        for kb in range(32):
                            kbs = slice(kb * 128, kb * 128 + 128)
                            ps_t = PS.tile([128, 1024], f32, tag="s")
                            for s in range(2):
                                qs = slice(qtb + s * 512, qtb + s * 512 + 512)
                                nc.tensor.matmul(
                                    ps_t[:, s * 512: s * 512 + 512],
                                    Kp[j][:, kbs], Qp[j][:, qs],
                                    start=True, stop=False)
                            for s in range(2):
                                qs = slice(qtb + s * 512, qtb + s * 512 + 512)
                                rr = slice(0, 64) if s == 0 else slice(64, 128)
                                nc.tensor.matmul(
                                    ps_t[:, s * 512: s * 512 + 512],
                                    ohkh_s[rr, kbs], RelHT[j][rr, qs],
                                    start=False, stop=True)
                            e_t = ES.tile([128, 1024], bf16, tag="e", bufs=9)
                            nc.scalar.activation(e_t[:, :], ps_t[:, :], AF.Exp)
                            pend.append((kb, e_t))
                            if len(pend) > 6:
                                pkb, pe_t = pend.pop(0)
                                for s in range(2):
                                    nc.tensor.matmul(
                                        ctx_ps[s][0:65, :],
                                        Vp[j][:, pkb * 65: pkb * 65 + 65],
                                        pe_t[:, s * 512: s * 512 + 512],
                                        start=(pkb == 0), stop=False)
                        for n, (pkb, pe_t) in enumerate(pend):
                            for s in range(2):
                                nc.tensor.matmul(
                                    ctx_ps[s][0:65, :],
                                    Vp[j][:, pkb * 65: pkb * 65 + 65],
                                    pe_t[:, s * 512: s * 512 + 512],
                                    start=(pkb == 0), stop=(pkb == 31))
                        for s in range(2):
                            den = ES.tile([65, 512], bf16, tag="den")
                            with nc.allow_low_precision(
                                    reason="softmax denom recip; common "
                                    "factor per column, 2^-9 rel err"):
                                nc.vector.reciprocal(den[64:65, :],
                                                     ctx_ps[s][64:65, :])
                            pb_t = ctx_ps[s][64:128, :]
                            nc.tensor.matmul(pb_t, ones_s[64:65, :],
                                             den[64:65, :], start=True,
                                             stop=True,
                                             tile_position=(64, 64))
                            bc_s = ES.tile([64, 512], f32, tag="bcs")
                            nc.vector.tensor_copy(bc_s[:, 0:256],
                                                  pb_t[:, 0:256])
                            nc.scalar.activation(bc_s[:, 256:512],
                                                 pb_t[:, 256:512], AF.Copy)
                            cdst = ctx_sb[uidx][:, qt * 1024 + s * 512:
                                               qt * 1024 + s * 512 + 512]
                            tmp = ES.tile([64, 512], f32, tag="tmp")
                            nc.vector.tensor_tensor(tmp[:, :],
                                                    ctx_ps[s][0:64, :],
                                                    bc_s[:, :], op=ALU.mult)
                            nc.vector.tensor_scalar_add(
                                cdst, tmp[:, :], vb_s[0:64, j: j + 1])

                def proj(half, tail, subs=(0, 1, 2, 3)):
                    us = [(ui, j) for ui, (j, hf) in enumerate(sched)
                          if hf == half]
                    po_f = [None]
                    for n, (ocb, s) in enumerate(
                            (o, q) for o in range(6) for q in subs):
                        if tail:
                            if n % 2 == 0:
                                po_f[0] = PS.tile([128, 1024], f32, tag="s",
                                                  name="pot")
                            po_t = po_f[0][:, (n % 2) * 512:
                                           (n % 2) * 512 + 512]
                        else:
                            po_t = PC.tile([128, 512], f32, tag="ctx",
                                           name="pot")
                        for i, (ui, j) in enumerate(us):
                            nc.tensor.matmul(
                                po_t[:, :],
                                projT_s[:, j * DIM + ocb * 128:
                                        j * DIM + ocb * 128 + 128],
                                ctx_sb[ui][:, s * 512: s * 512 + 512],
                                start=(i == 0), stop=(i == len(us) - 1))
                        po_s = POS.tile([128, 512], bf16, tag="pos")
                        if tail and (ocb + s) % 2 == 0:
                            nc.scalar.activation(po_s[:, :], po_t[:, :],
                                                 AF.Copy)
                        else:
                            nc.vector.tensor_copy(po_s[:, :], po_t[:, :])
                        nc.sync.dma_start(
                            outT_d[ocb * 128: ocb * 128 + 128,
                                   half * 2048 + s * 512:
                                   half * 2048 + s * 512 + 512],
                            po_s[:, :])

                attention_unit(0, *sched[0])
                attention_unit(1, *sched[1], qts=(0,))
                proj(1, tail=False, subs=(0, 1))
                attention_unit(1, *sched[1], qts=(1,))
                proj(1, tail=False, subs=(2, 3))
                attention_unit(2, *sched[2], qts=(0,))
                proj(0, tail=False, subs=(0, 1))
                attention_unit(2, *sched[2], qts=(1,))

            with tc.tile_pool(name="potail", bufs=2, space="PSUM") as PT, \
                 tc.tile_pool(name="postail", bufs=8) as PX:
                us = [(ui, j) for ui, (j, hf) in enumerate(sched) if hf == 0]
                for ocb in range(6):
                    # both remaining q-subtiles per weight slice: one
                    # LDWEIGHTS serves 2 matmuls on real hardware
                    po_t = [PT.tile([128, 512], f32, tag=f"po{_q}",
                                    name=f"pot{_q}") for _q in range(2)]
                    for i, (ui, j) in enumerate(us):
                        for qi, sq in enumerate((2, 3)):
                            nc.tensor.matmul(
                                po_t[qi][:, :],
                                projT_s[:, j * DIM + ocb * 128:
                                        j * DIM + ocb * 128 + 128],
                                ctx_sb[ui][:, sq * 512: sq * 512 + 512],
                                start=(i == 0), stop=(i == len(us) - 1))
                    for qi, sq in enumerate((2, 3)):
                        po_s = PX.tile([128, 512], bf16, tag="pos")
                        if (ocb + sq) % 2 == 0:
                            nc.scalar.activation(po_s[:, :], po_t[qi][:, :],
                                                 AF.Copy)
                        else:
                            nc.vector.tensor_copy(po_s[:, :], po_t[qi][:, :])
                        nc.sync.dma_start(
                            outT_d[ocb * 128: ocb * 128 + 128,
                                   sq * 512: sq * 512 + 512],
                            po_s[:, :])

    nc.compile()
    return nc


def kernel(x, qkv_w, qkv_b, proj_w, proj_b, rel_pos_h, rel_pos_w, num_heads):
    global LAST_EXEC_NS, _PROGRAM
    from concourse.bass_utils import run_bass_kernel_spmd

    x = np.asarray(x, dtype=np.float32)
    qkv_w = np.asarray(qkv_w, dtype=np.float32)
    qkv_b = np.asarray(qkv_b, dtype=np.float32)
    proj_w = np.asarray(proj_w, dtype=np.float32)
    proj_b = np.asarray(proj_b, dtype=np.float32)
    rel_pos_h = np.asarray(rel_pos_h, dtype=np.float32)
    rel_pos_w = np.asarray(rel_pos_w, dtype=np.float32)
    assert int(num_heads) == NH

    in_maps = [_prep_core_inputs(c, x, qkv_w, qkv_b, proj_w,
                                 rel_pos_h, rel_pos_w) for c in range(NCORES)]

    if _PROGRAM is None:
        _PROGRAM = _build_program()
    nc = _PROGRAM

    import os
    trace = os.environ.get("KERNEL_TRACE", "0") == "1"
    res = run_bass_kernel_spmd(nc, in_maps, core_ids=list(range(NCORES)),
                               trace=trace)
    LAST_EXEC_NS = res.exec_time_ns

    out = np.zeros((DIM, HW), dtype=np.float32)
    for c in range(NCORES):
        o = np.asarray(res.results[c]["outT"], dtype=np.float32)
        if c % 2 == 1:  # un-swap token halves
            o = np.concatenate([o[:, 2048:], o[:, :2048]], axis=1)
        out += o
    out = out.T + proj_b[None, :]
    return out.reshape(1, Hh, Ww, DIM).astype(np.float32)
